# revision 21
# baseline (speedup 1.0000x reference)
"""Gated Linear Attention (GLA) layer on 8 TRN2 NeuronCores.

Model: B=2, S=4096, D=2048, H=16 heads, DK=DV=128.
  q = (x@Wq)/sqrt(DK); k = x@Wk; v = x@Wv
  gk = log_sigmoid(x@Wg)/16
  S_t = diag(exp(gk_t)) S_{t-1} + k_t v_t^T;  o_t = q_t S_t
  out = o @ Wo

Sharding: core c handles batch b=c//4, head-group hg=c%4 (4 heads, 512 dims).
Each core computes its partial out = o_hg @ Wo[rows hg] in bf16; host sums
the 4 partials per batch in fp32 (the "all-reduce after Wo" done host-side).

Chunked-parallel recurrence (chunk C=128, inclusive in-chunk cumsum b of gk):
  qt = q*exp(b)/sqrt(DK), kt = k*exp(-b)  (relative decay within chunk)
  A^T = mask_{j<=i}( kt_j . qt_i );  oT = V^T A^T + S^T qtT
  S_exit = diag(exp(b_C)) (S_enter + kt^T V)

Performance structure (NTFF-measured 651 us/core on TRN2; PE-matmul busy
~607 us of that — the bf16 PE roofline for this op is ~594 us):
  - x is transposed+cast to bf16 on the HOST into 4-chunk "super-slabs"
    (t contiguous per k-tile), so the kernel does no x transposes and the
    q projection can run as N=512 matmuls.
  - q is projected directly in transposed per-head layout [dk, t] once per
    super-slab (LDW fully hidden under 512-wide streams); the within-chunk
    decay exp(bT) is applied per chunk from the per-head transposed cumsum
    bT (needed anyway for exp(b_C)).
  - k stays row-layout (needed for the state update) and is PE-transposed
    per head for the A^T matmul.
  - Software-pipelined emission: the recurrence+Wo of chunk n-1 is emitted
    interleaved ("pumped") between the projection matmuls of chunk n, so
    the in-order PE queue always has ready work; PSUM is planned at exactly
    8 banks (pr/pq/pw/pc pools x 2).
  - Startup is DMA-bandwidth-bound (12MB of weights+slab at ~360GB/s):
    weight/slab DMAs are split into 512KB quarters across the three
    SWDGE-capable queues (scalar/sync/gpsimd), ordered by first consumer;
    low-priority transfers (Wo, next slab) are issued outside the critical
    window.
  - A^T mask is a DVE multiply with a triangular 0/1 constant (frees ACT
    and gpsimd from the mask path).
  - All matmuls bf16 (PSUM accumulates fp32); partial outputs written bf16
    and summed in fp32 on the host.
"""
import numpy as np

B, S, D = 2, 4096, 2048
H, DK = 16, 128
HG = 4            # head-groups (cores per batch)
HPG = H // HG     # heads per group = 4
DG = HPG * DK     # 512 dims per group
C = 128           # time chunk
NCH = S // C      # 32 chunks
NKT = D // 128    # 16 k-tiles for projections
GATE_NORM = 16.0


def _split_waits(nc, mybir, cap=1):
    """Walrus codegen rejects >1 sync wait on some instruction structs
    (fused-LDW matmul, Drain). Move excess waits onto preceding single-wait
    NOPs on the same engine — engines are in-order so this is equivalent."""
    cnt = 0

    def fix_block(b):
        nonlocal cnt
        out = []
        changed = False
        for inst in list(b.instructions):
            si = getattr(inst, "sync_info", None)
            if si is not None and len(si.on_wait) > cap:
                waits = list(si.on_wait)
                for w in waits[:-cap]:
                    nop = mybir.InstNoOp(
                        name=f"I-swait-{cnt}", ins=[], outs=[], engine=inst.engine,
                        sync_info=mybir.SyncInfo(on_wait=[w], on_update=[]))
                    cnt += 1
                    out.append(nop)
                inst.sync_info = mybir.SyncInfo(
                    on_wait=waits[-cap:], on_update=list(si.on_update))
                changed = True
            out.append(inst)
        if changed:
            b.instructions = out

    def walk(b):
        fix_block(b)
        for sb in getattr(b, "blocks", []):
            walk(sb)

    for b in nc.m.functions[0].blocks:
        walk(b)


def _build(repeats=1):
    import concourse.bass as bass
    import concourse.mybir as mybir
    import concourse.tile as tile

    f32 = mybir.dt.float32
    bf16 = mybir.dt.bfloat16
    AF = mybir.ActivationFunctionType
    MUL = mybir.AluOpType.mult
    ADD = mybir.AluOpType.add

    nc = bass.Bass()
    NSUP = NCH // 4   # 4-chunk super-slabs of transposed x
    # xT layout: row r = m*128 + p, col kt*512 + t holds x[b, m*512 + t,
    # kt*128 + p] — each 128-row slab is a 4-chunk transposed bf16 block,
    # t contiguous per k-tile so the q projection can stream N=512.
    xT = nc.dram_tensor("xT", [NSUP * 128, NKT * 512], bf16,
                        kind="ExternalInput")
    # W[qkvg]: [128, NKT*DG] bf16, tile kt at cols [kt*DG, (kt+1)*DG) holds
    # W[kt*128:(kt+1)*128 rows of d_in, head-group cols].
    Wq = nc.dram_tensor("Wq", [128, NKT * DG], bf16, kind="ExternalInput")
    Wk = nc.dram_tensor("Wk", [128, NKT * DG], bf16, kind="ExternalInput")
    Wv = nc.dram_tensor("Wv", [128, NKT * DG], bf16, kind="ExternalInput")
    Wg = nc.dram_tensor("Wg", [128, NKT * DG], bf16, kind="ExternalInput")
    # Wo: [128, HPG*D] bf16; head h at cols [h*D, (h+1)*D).
    Wo = nc.dram_tensor("Wo", [128, HPG * D], bf16, kind="ExternalInput")
    Mc = nc.dram_tensor("Mc", [C, C], bf16, kind="ExternalInput")   # -1/16 triu
    Um = nc.dram_tensor("Um", [C, C], bf16, kind="ExternalInput")   # 0/1 triu
    Id = nc.dram_tensor("Id", [128, 128], bf16, kind="ExternalInput")
    out = nc.dram_tensor("out", [S, D], bf16, kind="ExternalOutput")

    with tile.TileContext(nc) as tc:
        with tc.tile_pool(name="const", bufs=1) as cpool, \
             tc.tile_pool(name="sb", bufs=2) as sb, \
             tc.tile_pool(name="st", bufs=1) as st, \
             tc.tile_pool(name="pr", bufs=2, space="PSUM") as pr, \
             tc.tile_pool(name="pq", bufs=2, space="PSUM") as pq, \
             tc.tile_pool(name="pw", bufs=2, space="PSUM") as pw, \
             tc.tile_pool(name="pc", bufs=2, space="PSUM") as pc:

            # ---- constants / weights ----
            # Weight loads gate chunk 0: split each 2MB weight into 512KB
            # quarters spread over the SWDGE-capable queues (scalar/sync/
            # gpsimd), ordered by first consumer, so projections unblock
            # progressively (sub-tile deps) instead of after a full 2MB.
            w_tiles = {n: cpool.tile([128, NKT * DG], bf16, name=f"W{n}_sb")
                       for n in ("q", "k", "v", "g")}
            Wo_sb = cpool.tile([128, HPG * D], bf16, name="Wo_sb")
            M_sb = cpool.tile([C, C], bf16, name="M_sb")
            U_sb = cpool.tile([C, C], bf16, name="U_sb")
            ident = cpool.tile([128, 128], bf16, name="ident")
            nc.gpsimd.dma_start(out=M_sb, in_=Mc[:, :])
            nc.gpsimd.dma_start(out=U_sb, in_=Um[:, :])
            nc.gpsimd.dma_start(out=ident, in_=Id[:, :])
            QW = NKT * DG // 4  # quarter width
            for p in range(4):
                qs = slice(p * QW, (p + 1) * QW)
                nc.scalar.dma_start(out=w_tiles["g"][:, qs], in_=Wg[:, qs])
                nc.sync.dma_start(out=w_tiles["q"][:, qs], in_=Wq[:, qs])
            for p in range(4):
                qs = slice(p * QW, (p + 1) * QW)
                nc.scalar.dma_start(out=w_tiles["k"][:, qs], in_=Wk[:, qs])
                nc.sync.dma_start(out=w_tiles["v"][:, qs], in_=Wv[:, qs])

            ln_qscale = float(np.log(DK ** -0.5))
            lnq_bias = cpool.tile([128, 1], f32, name="lnq_bias")
            nc.vector.memset(lnq_bias, ln_qscale)

            # ---- per-head recurrent state ----
            S_st = [st.tile([DK, DK], f32, name=f"S{h}") for h in range(HPG)]
            S_bf = [st.tile([DK, DK], bf16, name=f"Sb{h}") for h in range(HPG)]
            for h in range(HPG):
                nc.vector.memset(S_st[h], 0.0)
                nc.vector.memset(S_bf[h], 0.0)

            def rec_stream(ctx):
                """Yield-between-instructions emitter for the recurrence +
                Wo of one chunk. ctx holds that chunk's tiles."""
                t0 = ctx["t0"]
                oT_sb = sb.tile([128, DG], bf16, name="oT_sb")
                for h in range(HPG):
                    hs = slice(h * DK, (h + 1) * DK)
                    # A^T[j,i] = kt_j . qt_i , masked to j<=i
                    at_ps = pc.tile([C, C], f32, name="at", tag="c")
                    nc.tensor.matmul(at_ps, ctx["ktT"][h], ctx["qtT"][h],
                                     start=True, stop=True)
                    yield
                    AT_m = sb.tile([C, C], bf16, name="AT_m")
                    nc.vector.tensor_tensor(out=AT_m, in0=at_ps, in1=U_sb,
                                            op=MUL)
                    yield
                    # oT = V^T A^T + S^T qtT   ([dv, t])
                    oT_ps = pc.tile([DK, C], f32, name="oT", tag="c")
                    nc.tensor.matmul(oT_ps, ctx["v"][:, hs], AT_m,
                                     start=True, stop=False)
                    yield
                    nc.tensor.matmul(oT_ps, S_bf[h], ctx["qtT"][h],
                                     start=False, stop=True)
                    yield
                    nc.scalar.copy(oT_sb[:, hs], oT_ps)
                    if ctx.get("last"):
                        yield
                        continue  # exiting state is dead on the last chunk
                    # state update: S = exp(b_C) * (S + kt^T V)
                    st_ps = pc.tile([DK, DK], f32, name="st", tag="c")
                    nc.tensor.matmul(st_ps, ctx["kt"][:, hs], ctx["v"][:, hs],
                                     start=True, stop=True)
                    yield
                    nc.vector.tensor_tensor(out=S_st[h], in0=S_st[h],
                                            in1=st_ps, op=ADD)
                    yield
                    nc.vector.tensor_scalar(
                        out=S_st[h], in0=S_st[h],
                        scalar1=ctx["ebC"][:, h:h + 1], scalar2=None, op0=MUL)
                    nc.scalar.copy(S_bf[h], S_st[h])
                    yield
                # partial out chunk = o_hg @ Wo_rows  (bf16); DMA each
                # 512-col slice as soon as its copy lands so the last
                # chunk's store isn't one big exposed transfer.
                out_sb = sb.tile([128, D], bf16, name="out_sb")
                for ns in range(4):
                    op_ps = pw.tile([128, 512], f32, name="op", tag="w")
                    for h in range(HPG):
                        nc.tensor.matmul(
                            op_ps, oT_sb[:, h * DK:(h + 1) * DK],
                            Wo_sb[:, h * D + ns * 512: h * D + (ns + 1) * 512],
                            start=(h == 0), stop=(h == HPG - 1))
                        yield
                    nc.vector.tensor_copy(out_sb[:, ns * 512:(ns + 1) * 512],
                                          op_ps)
                    yield
                    nc.sync.dma_start(
                        out=out[t0:t0 + 128, ns * 512:(ns + 1) * 512],
                        in_=out_sb[:, ns * 512:(ns + 1) * 512])

            stream = None

            def pump(n=1):
                if stream is not None:
                    for _ in range(n):
                        next(stream, None)

            for rep in range(repeats):
              slab_ring = {}
              SQ = NKT * 512 // 4

              def load_slab(m, token=None):
                  # quarter-split: consumers unblock per 512KB piece instead
                  # of after the full 2MB (sub-tile deps are DMA-granular).
                  # dma_start only POSTS descriptors — transfers run
                  # concurrently with everything else regardless of emission
                  # order — so a prefetch gets a dummy WAW dependency (token
                  # write into the tile) to keep it out of the startup
                  # bandwidth melee until real compute exists.
                  t = sb.tile([128, NKT * 512], bf16, name="slab")
                  r = slice(m * 128, (m + 1) * 128)
                  for p in range(4):
                      cs = slice(p * SQ, (p + 1) * SQ)
                      nc.gpsimd.dma_start(out=t[:, cs], in_=xT[r, cs])
                  slab_ring[m] = t

              load_slab(0)
              for ci in range(NCH + 1):
                cur = None
                if ci < NCH:
                    t0 = ci * C
                    m, sc = divmod(ci, 4)
                    slab = slab_ring[m]
                    if ci == 1:
                        nc.gpsimd.dma_start(out=Wo_sb, in_=Wo[:, :])
                    if sc == 1 and m + 1 < NSUP:
                        load_slab(m + 1)  # prefetch one super ahead
                    if sc == 0:
                        slab_ring.pop(m - 1, None)

                    def xsl(kt):
                        return slab[:, kt * 512 + sc * 128:
                                    kt * 512 + (sc + 1) * 128]

                    # g projection (row layout) + gate chain
                    g_ps = pr.tile([128, DG], f32, name="g_ps", tag="r")
                    for kt in range(NKT):
                        nc.tensor.matmul(
                            g_ps, xsl(kt),
                            w_tiles["g"][:, kt * DG:(kt + 1) * DG],
                            start=(kt == 0), stop=(kt == NKT - 1))
                        pump()
                    # s = softplus(-z) = ln(1 + exp(-z))
                    emz = sb.tile([128, DG], bf16, name="emz")
                    nc.scalar.activation(emz, g_ps, AF.Exp, scale=-1.0)
                    s_all = sb.tile([128, DG], bf16, name="s_all")
                    nc.scalar.activation(s_all, emz, AF.Ln, bias=1.0)

                    if sc == 0:
                        # super q projection: qT[dk, 4-chunk t] per head,
                        # N=512 streams so LDW is fully hidden. After g so
                        # chunk 0 starts on 1MB of DMA, not 4MB.
                        qsup = []
                        for h in range(HPG):
                            qps = pq.tile([128, 512], f32, name="qps",
                                          tag="q")
                            for kt in range(NKT):
                                nc.tensor.matmul(
                                    qps,
                                    w_tiles["q"][:, kt * DG + h * DK:
                                                 kt * DG + (h + 1) * DK],
                                    slab[:, kt * 512:(kt + 1) * 512],
                                    start=(kt == 0), stop=(kt == NKT - 1))
                                pump()
                            qs = sb.tile([128, 512], bf16, name=f"qs{h}")
                            nc.vector.tensor_copy(qs, qps)
                            qsup.append(qs)
                        cur_qsup = qsup

                    # k projection (row layout)
                    k_ps = pr.tile([128, DG], f32, name="k_ps", tag="r")
                    for kt in range(NKT):
                        nc.tensor.matmul(
                            k_ps, xsl(kt),
                            w_tiles["k"][:, kt * DG:(kt + 1) * DG],
                            start=(kt == 0), stop=(kt == NKT - 1))
                        pump()

                    # cumsum b (row layout, for enb) and per-head bT
                    b_ps = pq.tile([128, DG], f32, name="b_ps", tag="q")
                    nc.tensor.matmul(b_ps, M_sb, s_all, start=True, stop=True)
                    pump()
                    enb = sb.tile([128, DG], f32, name="enb")
                    nc.scalar.activation(enb, b_ps, AF.Exp, scale=-1.0)
                    ebC = sb.tile([128, HPG], f32, name="ebC")
                    ebqT = sb.tile([128, HPG * C], bf16, name="ebqT")
                    for h in range(HPG):
                        bT = pc.tile([DK, C], f32, name="bT", tag="c")
                        nc.tensor.matmul(bT, s_all[:, h * DK:(h + 1) * DK],
                                         M_sb, start=True, stop=True)
                        nc.scalar.activation(ebC[:, h:h + 1],
                                             bT[:, C - 1:C], AF.Exp)
                        nc.scalar.activation(ebqT[:, h * C:(h + 1) * C],
                                             bT, AF.Exp, bias=lnq_bias)
                        pump()

                    # v projection (row layout)
                    v_ps = pr.tile([128, DG], f32, name="v_ps", tag="r")
                    for kt in range(NKT):
                        nc.tensor.matmul(
                            v_ps, xsl(kt),
                            w_tiles["v"][:, kt * DG:(kt + 1) * DG],
                            start=(kt == 0), stop=(kt == NKT - 1))
                        pump()
                    v_sb = sb.tile([128, DG], bf16, name="v_sb")
                    nc.scalar.copy(v_sb, v_ps)

                    # scaled q (transposed) and k (row); kT per head via PE
                    kt_all = sb.tile([128, DG], bf16, name="kt_all")
                    nc.vector.tensor_tensor(out=kt_all, in0=k_ps, in1=enb,
                                            op=MUL)
                    qtT = []
                    for h in range(HPG):
                        q_sb = sb.tile([DK, C], bf16, name=f"qtT{h}")
                        nc.vector.tensor_tensor(
                            out=q_sb,
                            in0=cur_qsup[h][:, sc * 128:(sc + 1) * 128],
                            in1=ebqT[:, h * C:(h + 1) * C], op=MUL)
                        qtT.append(q_sb)
                    ktT = []
                    for h in range(HPG):
                        tp = pc.tile([128, 128], bf16, name="tp_k", tag="c")
                        nc.tensor.transpose(
                            tp, kt_all[:, h * DK:(h + 1) * DK], ident)
                        pump()
                        k_sb = sb.tile([DK, C], bf16, name=f"ktT{h}")
                        nc.vector.tensor_copy(k_sb, tp)
                        ktT.append(k_sb)

                    cur = {"t0": t0, "qtT": qtT, "ktT": ktT, "kt": kt_all,
                           "v": v_sb, "ebC": ebC, "last": ci == NCH - 1}

                # drain the previous chunk's remaining recurrence + Wo
                pump(200)
                stream = rec_stream(cur) if cur is not None else None

    _split_waits(nc, mybir)
    return nc


_NC_CACHE = None


def _make_in_maps(x, Wq, Wk, Wv, Wg, Wo):
    import ml_dtypes

    bf16 = ml_dtypes.bfloat16
    x = np.asarray(x, dtype=np.float32)
    Ws = [np.asarray(w, dtype=np.float32) for w in (Wq, Wk, Wv, Wg)]
    Wo = np.asarray(Wo, dtype=np.float32)

    Mc = np.triu(np.full((C, C), -1.0 / GATE_NORM, np.float32)).astype(bf16)
    Um = np.triu(np.ones((C, C), np.float32)).astype(bf16)
    Id = np.eye(128, dtype=np.float32).astype(bf16)

    # host-side transpose+cast of x into 4-chunk super-slabs: row m*128+p,
    # col kt*512+t holds x[b, m*512+t, kt*128+p]
    NSUP = NCH // 4
    xTs = []
    for b in range(B):
        a = x[b].reshape(NSUP, 512, NKT, 128).transpose(0, 3, 2, 1)
        xTs.append(np.ascontiguousarray(a).astype(bf16).reshape(
            NSUP * 128, NKT * 512))

    def tile_w(w):  # [2048, 512] -> [128, 16*512]
        return np.ascontiguousarray(
            w.reshape(NKT, 128, DG).transpose(1, 0, 2).reshape(128, NKT * DG)
        ).astype(bf16)

    in_maps = []
    for c in range(8):
        b, hg = c // 4, c % 4
        cols = slice(hg * DG, (hg + 1) * DG)
        wo = Wo[cols, :].reshape(HPG, 128, D).transpose(1, 0, 2)
        m = {"xT": xTs[b],
             "Wo": np.ascontiguousarray(wo.reshape(128, HPG * D)).astype(bf16),
             "Mc": Mc, "Um": Um, "Id": Id}
        for name, w in zip(("Wq", "Wk", "Wv", "Wg"), Ws):
            m[name] = tile_w(w[:, cols])
        in_maps.append(m)
    return in_maps


def kernel(x, Wq, Wk, Wv, Wg, Wo):
    global _NC_CACHE
    from concourse import bass_utils

    in_maps = _make_in_maps(x, Wq, Wk, Wv, Wg, Wo)

    if _NC_CACHE is None:
        _NC_CACHE = _build()
    r = bass_utils.run_bass_kernel_spmd(_NC_CACHE, in_maps, core_ids=list(range(8)))

    res = np.zeros((B, S, D), dtype=np.float32)
    for c in range(8):
        res[c // 4] += r.results[c]["out"].astype(np.float32)
    return res


def bench(inputs, iters=20, warmup=4):
    """Marginal per-NEFF-execution wall time (ns) with device-resident
    inputs and pipelined dispatch, mirroring bass2jax.run_bass_via_pjrt's
    shard_map construction (donated zero output buffers, pre-staged)."""
    global _NC_CACHE
    import time as _time

    import jax
    import jax.numpy as jnp
    from jax.experimental.shard_map import shard_map
    from jax.sharding import Mesh, NamedSharding, PartitionSpec

    from concourse import bass2jax, mybir

    if _NC_CACHE is None:
        _NC_CACHE = _build()
    nc = _NC_CACHE
    bass2jax.install_neuronx_cc_hook()

    in_maps = _make_in_maps(**inputs)

    partition_name = nc.partition_id_tensor.name if nc.partition_id_tensor else None
    in_names, out_names, out_avals = [], [], []
    for alloc in nc.m.functions[0].allocations:
        if not isinstance(alloc, mybir.MemoryLocationSet):
            continue
        name = alloc.memorylocations[0].name
        if alloc.kind == "ExternalInput":
            if name != partition_name:
                in_names.append(name)
        elif alloc.kind == "ExternalOutput":
            out_names.append(name)
            out_avals.append(
                jax.core.ShapedArray(
                    tuple(alloc.tensor_shape), mybir.dt.np(alloc.dtype)))
    n_params = len(in_names)
    n_outs = len(out_names)
    in_names = in_names + out_names
    if partition_name is not None:
        in_names.append(partition_name)

    def _body(*args):
        operands = list(args)
        if partition_name is not None:
            operands.append(bass2jax.partition_id_tensor())
        outs = bass2jax._bass_exec_p.bind(
            *operands,
            out_avals=tuple(out_avals),
            in_names=tuple(in_names),
            out_names=tuple(out_names),
            lowering_input_output_aliases=(),
            sim_require_finite=True,
            sim_require_nnan=True,
            nc=nc,
        )
        return tuple(outs)

    n_cores = 8
    devices = jax.devices()[:n_cores]
    mesh = Mesh(np.asarray(devices), ("core",))
    in_specs = (PartitionSpec("core"),) * (n_params + n_outs)
    out_specs = (PartitionSpec("core"),) * n_outs
    donate = tuple(range(n_params, n_params + n_outs))
    f = jax.jit(
        shard_map(_body, mesh=mesh, in_specs=in_specs, out_specs=out_specs,
                  check_rep=False),
        donate_argnums=donate, keep_unused=True)

    sh = NamedSharding(mesh, PartitionSpec("core"))
    concat_in = [
        jax.device_put(
            np.concatenate(
                [np.asarray(in_maps[c][name]) for c in range(n_cores)], axis=0),
            sh)
        for name in in_names[:n_params]
    ]
    mk_zeros = jax.jit(
        lambda: tuple(
            jnp.zeros((n_cores * a.shape[0], *a.shape[1:]), a.dtype)
            for a in out_avals),
        out_shardings=tuple(sh for _ in out_avals))

    zsets = [mk_zeros() for _ in range(warmup + iters)]
    for zs in zsets:
        for z in zs:
            z.block_until_ready()

    last = None
    for i in range(warmup):
        last = f(*concat_in, *zsets[i])
    for o in last:
        o.block_until_ready()

    # single-call latency
    t0 = _time.perf_counter()
    last = f(*concat_in, *zsets[warmup])
    for o in last:
        o.block_until_ready()
    t_single = _time.perf_counter() - t0

    # pipelined marginal time
    t0 = _time.perf_counter()
    for i in range(warmup + 1, warmup + iters):
        last = f(*concat_in, *zsets[i])
    for o in last:
        o.block_until_ready()
    t_marginal = (_time.perf_counter() - t0) / (iters - 1)

    print(f"bench: single-call {t_single * 1e6:.1f} us, "
          f"pipelined marginal {t_marginal * 1e6:.1f} us over {iters - 1} iters")
    return t_marginal * 1e9